# revision 44
# baseline (speedup 1.0000x reference)
"""Trainium2 Bass kernel: float32 -> 32-channel bit-plane encoding.

For input x [4096, 512] f32, produces out [4096, 512, 32] f32 where
out[b, f, 0] = (x[b,f] < 0) and out[b, f, 1+j] = bit (30-j) of
bitcast_int32(|x[b,f]|), MSB first.

Wire-format design: every output element is exactly 0.0 or 1.0, so the
device computes and stores each of the 67M output elements as a uint8
{0,1}; the host applies a value-preserving widening cast to f32.  This
cuts device HBM write traffic 4x (8MB/core instead of 32MB/core), which
is the binding roofline.

Host-side repack makes the device compute uniform:
  i' = (bitcast_u32(x) & 0x7FFFFFFF) | ((x < 0) << 31)
stored as a big-endian byte stream, viewed as uint16 pairs.  Then output
channel k of feature f equals bit (7 - k%8) of stream byte 4f + k//8.

Device compute (VectorE), one fused tensor_scalar op per bit plane:
  plane_m = (x_u16 >> (7-m)) & 0x0101     m = 0..7
Each uint16 element yields TWO planar output bytes; the dense step-1
16-bit single-src pattern hits the DVE 4x perf mode (~602ns steady-state
per FD=2048 op, ~850GB/s production).

Schedule (trace-derived, iterated over ~10 hardware profiles):
  - Concurrent output DMA is throttled to ~310-470GB/s while the DVE
    streams (SBUF port contention) and bursts to ~750-830GB/s after
    the last DVE op, so output pieces are gated as early as each
    plane completes to overlap the throttled phase, while the piece
    count stays at 5 (per-DMA ring/completion overhead is ~1us, and
    finer pieces measurably lower the achieved rate).
  - Queue concurrency does not add bandwidth (the mover pool/AXI port
    is shared and interleaving hurts), so all output pieces ride the
    SP HWDGE queue serially; the ring never empties (each next piece
    is pre-issued), avoiding the ~2.6us cold-restart.
  - Inputs: rt0 on the SP ring, rt1 on the Act ring (both
    first-on-ring, receipt ~2.6-3.3us after issue regardless of size;
    splitting them makes receipt later, not earlier), rt2/rt3 on the
    GpSimd SWDGE queue.  Section 0 planes 0-1 are computed in rt0/rt1
    halves so DVE starts on whichever input lands first.
  - Each piece writes its own dram tensor (contiguous DRAM range).
    GpSimd SWDGE must not carry output pieces (~77GB/s there), and
    the Pool engine cannot run shift/and tensor_scalar ops.

The planes land in HBM planar per SBUF partition; the host reassembles
pieces and interleaves planes/sections into [rows, F, 32] during the
f32 cast.

Sharded row-wise over 8 NeuronCores (512 rows each, 4 row tiles of 128).
"""

import sys

if "/opt/trn_rl_repo" not in sys.path:
    sys.path.insert(0, "/opt/trn_rl_repo")

import numpy as np

import concourse.bass as bass
import concourse.mybir as mybir

P = 128           # SBUF partitions
F = 512           # features per row
K = 32            # output channels per feature
N_CORES = 8
ROWS_TOTAL = 4096
ROWS = ROWS_TOTAL // N_CORES   # rows per core (512)
NRT = ROWS // P                # row tiles per core (4)
W16 = F * 2                    # uint16 words per row (1024)
HW16 = W16 // 2                # half-row-tile input chunk (512)
PW = 2 * W16                   # pair-section width (2048 u16)
PLANES = 8                     # bit planes per byte
OWS = PLANES * PW              # output u16 per partition per section
OCOLS = 2 * OWS                # output dram columns per partition (32768)
SPLIT = PW - 256               # last-plane split point (u16 cols; the
                               # final op is FD=256 so the last output
                               # piece's gate lands as early as possible)

# out pieces: (engine 0=sync/1=scalar, sec, u16 lo, hi, ts gate,
# pool gate); see build_nc for rationale.  Module-level because the
# host-side unpack reassembles the per-piece dram tensors using the
# same table.
PIECES = [
    (0, 0, 0, 1 * PW, 3, 0),                 # sec0 p0 (512KB)
    (0, 0, 1 * PW, 4 * PW, 7, 0),            # sec0 p1-p3 (1.5MB)
    (0, 0, 4 * PW, 8 * PW, 10, 0),           # sec0 p4-p7 (2MB)
    (0, 1, 0, 4 * PW, 14, 0),                # sec1 p0-p3 (2MB)
    (0, 1, 4 * PW, 8 * PW, 18, 0),           # sec1 p4-p7 (2MB)
]


def build_nc() -> bass.Bass:
    nc = bass.Bass("TRN2", target_bir_lowering=False, debug=False)
    u16 = mybir.dt.uint16

    xin = nc.declare_dram_parameter("xin", [ROWS, W16], u16, isOutput=False)
    xin_ap = xin.ap()
    # one dram tensor per output piece: piece k's 128 partition chunks
    # are adjacent rows -> one contiguous DRAM range per piece
    piece_out = [
        nc.declare_dram_parameter(f"out{k}", [P, pc[3] - pc[2]], u16,
                                  isOutput=True)
        for k, pc in enumerate(PIECES)
    ]
    piece_out_ap = [t.ap() for t in piece_out]

    shift_and = (mybir.AluOpType.logical_shift_right,
                 mybir.AluOpType.bitwise_and)

    # vector ops: (section, m, lo, hi), ts count = index+1.
    # Section 0 planes 0-1 are split at the rt0/rt1 boundary: the two
    # rt0 halves (gated on in0 alone) absorb receipt divergence between
    # the two input rings, and plane 0 completes at ts3 so the first
    # output piece can go early.
    vops = [(0, 0, 0, W16),                                  # ts 1 (rt0)
            (0, 1, 0, W16),                                  # ts 2 (rt0)
            (0, 0, W16, PW),                                 # ts 3 (rt1)
            (0, 1, W16, PW)]                                 # ts 4 (rt1)
    vops += [(0, m, 0, PW) for m in range(2, PLANES)]        # ts 5..10
    vops += [(1, m, 0, PW) for m in range(PLANES)]           # ts 11..18
    # Queue concurrency does not add bandwidth (HBM write interleaving
    # thrashes); fewer/bigger pieces stream faster, and the SP queue
    # consistently outruns the Act queue (~2x), so all pieces ride the
    # SP queue serially with the ring kept hot (next piece pre-issued).

    from contextlib import ExitStack
    with ExitStack() as ctx:
        xt = [ctx.enter_context(nc.sbuf_tensor(f"xt{s}", [P, PW], u16))
              for s in range(2)]
        ot = [ctx.enter_context(nc.sbuf_tensor(f"ot{s}", [P, OWS], u16))
              for s in range(2)]

        in_sem = [ctx.enter_context(nc.semaphore(f"in_sem{b}"))
                  for b in range(NRT + 1)]   # rt0lo, rt0hi, rt1, rt2, rt3
        ts_sem = ctx.enter_context(nc.semaphore("ts_sem"))
        pts_sem = ctx.enter_context(nc.semaphore("pts_sem"))
        od_sem = ctx.enter_context(nc.semaphore("od_sem"))

        ctx.enter_context(nc.Block(no_gpsimd_drain=True))
        block = nc.cur_block

        @block.vector
        def _(vec: bass.BassEngine):
            for i, (sec, m, lo, hi) in enumerate(vops):
                if i == 0:
                    vec.wait_ge(in_sem[0], 16)
                    vec.wait_ge(in_sem[1], 16)
                elif i == 2:
                    vec.wait_ge(in_sem[2], 16)
                elif i == PLANES + 2:
                    vec.wait_ge(in_sem[3], 16)
                    vec.wait_ge(in_sem[4], 16)
                vec.tensor_scalar(
                    ot[sec][:, m * PW + lo:m * PW + hi],
                    xt[sec][:, lo:hi],
                    7 - m,
                    0x0101,
                    *shift_and,
                ).then_inc(ts_sem)

        def piece_dma(eng, k):
            _, sec, lo, hi, n, pn = PIECES[k]
            eng.wait_ge(ts_sem, n)
            if pn:
                eng.wait_ge(pts_sem, pn)
            eng.dma_start(
                piece_out_ap[k][:, :],
                ot[sec][:, lo:hi],
            ).then_inc(od_sem, 16)

        @block.sync
        def _(sp: bass.BassEngine):
            # rt0 partitions 0-63 (128KB, first on SP ring): splitting
            # rt0 across BOTH HWDGE rings by partition halves halves the
            # transfer time of the gating first input
            sp.dma_start(xt[0][0:P // 2, 0:W16],
                         xin_ap[0:P // 2, :]).then_inc(in_sem[0], 16)
            for k, pc in enumerate(PIECES):
                if pc[0] == 0:
                    piece_dma(sp, k)

        @block.scalar
        def _(sc: bass.BassEngine):
            # rt0 partitions 64-127 (128KB, first on Act ring)
            sc.dma_start(xt[0][P // 2:P, 0:W16],
                         xin_ap[P // 2:P, :]).then_inc(in_sem[1], 16)
            for k, pc in enumerate(PIECES):
                if pc[0] == 1:
                    piece_dma(sc, k)

        @block.gpsimd
        def _(gp: bass.BassEngine):
            # rt1-rt3 via SWDGE, keeping the HWDGE rings free for the
            # gating rt0 halves and the output pieces
            gp.dma_start(xt[0][:, W16:PW],
                         xin_ap[P:2 * P, :]).then_inc(in_sem[2], 16)
            gp.dma_start(xt[1][:, 0:W16],
                         xin_ap[2 * P:3 * P, :]).then_inc(in_sem[3], 16)
            gp.dma_start(xt[1][:, W16:PW],
                         xin_ap[3 * P:4 * P, :]).then_inc(in_sem[4], 16)

    return nc


_NC_CACHE = None


def _get_nc():
    global _NC_CACHE
    if _NC_CACHE is None:
        _NC_CACHE = build_nc()
    return _NC_CACHE


def pack_shard(x_shard: np.ndarray) -> np.ndarray:
    """[ROWS, F] f32 -> [ROWS, W16] uint16: sign-normalized bitcast words
    as a big-endian byte stream, viewed as little-endian uint16 pairs."""
    x_shard = np.ascontiguousarray(x_shard)
    xi = (x_shard.view(np.uint32) & np.uint32(0x7FFFFFFF)) | \
        ((x_shard < 0).astype(np.uint32) << np.uint32(31))
    return xi.byteswap().view(np.uint16)


def unpack_shard(res_map: dict) -> np.ndarray:
    """Per-piece dram tensors -> [ROWS, F, K] f32.

    Pieces are reassembled into the planar pair-section layout
    [P, OCOLS]; section s covers row tiles (2s, 2s+1): bytes [p, sec,
    m, rt_in_pair, 4f+j] -> out[(2*sec+rt)*128+p, f, 8j+m].
    """
    raw = np.empty((P, OCOLS), dtype=np.uint16)
    for k, (eng, sec, lo, hi, n, pn) in enumerate(PIECES):
        raw[:, sec * OWS + lo:sec * OWS + hi] = res_map[f"out{k}"]
    b = raw.view(np.uint8).reshape(P, 2, PLANES, 2, F, 4)
    r = b.transpose(1, 3, 0, 4, 5, 2).reshape(ROWS, F, K)
    return r.astype(np.float32)


def kernel(x: np.ndarray) -> np.ndarray:
    from concourse.bass_utils import run_bass_kernel_spmd

    x = np.asarray(x, dtype=np.float32)
    assert x.shape == (ROWS_TOTAL, F), x.shape
    nc = _get_nc()
    in_maps = [
        {"xin": pack_shard(x[i * ROWS:(i + 1) * ROWS])} for i in range(N_CORES)
    ]
    res = run_bass_kernel_spmd(nc, in_maps, list(range(N_CORES)))
    parts = [unpack_shard(res.results[i]) for i in range(N_CORES)]
    return np.concatenate(parts, axis=0)


# revision 45
# speedup vs baseline: 1.1529x; 1.1529x over previous
"""Trainium2 Bass kernel: float32 -> 32-channel bit-plane encoding.

For input x [4096, 512] f32, produces out [4096, 512, 32] f32 where
out[b, f, 0] = (x[b,f] < 0) and out[b, f, 1+j] = bit (30-j) of
bitcast_int32(|x[b,f]|), MSB first.

Wire-format design: every output element is exactly 0.0 or 1.0, so the
device computes and stores each of the 67M output elements as a uint8
{0,1}; the host applies a value-preserving widening cast to f32.  This
cuts device HBM write traffic 4x (8MB/core instead of 32MB/core), which
is the binding roofline.

Host-side repack makes the device compute uniform:
  i' = (bitcast_u32(x) & 0x7FFFFFFF) | ((x < 0) << 31)
stored as a big-endian byte stream, viewed as uint16 pairs.  Then output
channel k of feature f equals bit (7 - k%8) of stream byte 4f + k//8.

Device compute (VectorE), one fused tensor_scalar op per bit plane:
  plane_m = (x_u16 >> (7-m)) & 0x0101     m = 0..7
Each uint16 element yields TWO planar output bytes; the dense step-1
16-bit single-src pattern hits the DVE 4x perf mode (~602ns steady-state
per FD=2048 op, ~850GB/s production).

Schedule (trace-derived, iterated over ~10 hardware profiles):
  - Concurrent output DMA is throttled to ~310-470GB/s while the DVE
    streams (SBUF port contention) and bursts to ~750-830GB/s after
    the last DVE op, so output pieces are gated as early as each
    plane completes to overlap the throttled phase, while the piece
    count stays at 5 (per-DMA ring/completion overhead is ~1us, and
    finer pieces measurably lower the achieved rate).
  - Queue concurrency does not add bandwidth (the mover pool/AXI port
    is shared and interleaving hurts), so all output pieces ride the
    SP HWDGE queue serially; the ring never empties (each next piece
    is pre-issued), avoiding the ~2.6us cold-restart.
  - Inputs: rt0 is split by PARTITION halves across the SP and Act
    rings (both first-on-ring and in parallel, halving the gating
    first transfer); rt1-rt3 ride the GpSimd SWDGE queue (plenty of
    slack before their planes).  Splitting an input by columns on ONE
    ring makes receipt later, not earlier (~2.6-3.3us per-DMA ring
    receipt latency regardless of size).  Section 0 planes 0-1 are
    computed in rt0/rt1 column halves so DVE starts as soon as rt0
    lands.
  - Each piece writes its own dram tensor (contiguous DRAM range).
    GpSimd SWDGE must not carry output pieces (~77GB/s there), and
    the Pool engine cannot run shift/and tensor_scalar ops.

The planes land in HBM planar per SBUF partition; the host reassembles
pieces and interleaves planes/sections into [rows, F, 32] during the
f32 cast.

Sharded row-wise over 8 NeuronCores (512 rows each, 4 row tiles of 128).
"""

import sys

if "/opt/trn_rl_repo" not in sys.path:
    sys.path.insert(0, "/opt/trn_rl_repo")

import numpy as np

import concourse.bass as bass
import concourse.mybir as mybir

P = 128           # SBUF partitions
F = 512           # features per row
K = 32            # output channels per feature
N_CORES = 8
ROWS_TOTAL = 4096
ROWS = ROWS_TOTAL // N_CORES   # rows per core (512)
NRT = ROWS // P                # row tiles per core (4)
W16 = F * 2                    # uint16 words per row (1024)
HW16 = W16 // 2                # half-row-tile input chunk (512)
PW = 2 * W16                   # pair-section width (2048 u16)
PLANES = 8                     # bit planes per byte
OWS = PLANES * PW              # output u16 per partition per section
OCOLS = 2 * OWS                # output dram columns per partition (32768)
SPLIT = PW - 256               # last-plane split point (u16 cols; the
                               # final op is FD=256 so the last output
                               # piece's gate lands as early as possible)

# out pieces: (engine 0=sync/1=scalar, sec, u16 lo, hi, ts gate,
# pool gate); see build_nc for rationale.  Module-level because the
# host-side unpack reassembles the per-piece dram tensors using the
# same table.
PIECES = [
    (0, 0, 0, 1 * PW, 3, 0),                 # sec0 p0 (512KB)
    (0, 0, 1 * PW, 4 * PW, 7, 0),            # sec0 p1-p3 (1.5MB)
    (0, 0, 4 * PW, 8 * PW, 10, 0),           # sec0 p4-p7 (2MB)
    (0, 1, 0, 4 * PW, 14, 0),                # sec1 p0-p3 (2MB)
    (0, 1, 4 * PW, 8 * PW, 18, 0),           # sec1 p4-p7 (2MB)
]


def build_nc() -> bass.Bass:
    nc = bass.Bass("TRN2", target_bir_lowering=False, debug=False)
    u16 = mybir.dt.uint16

    xin = nc.declare_dram_parameter("xin", [ROWS, W16], u16, isOutput=False)
    xin_ap = xin.ap()
    # one dram tensor per output piece: piece k's 128 partition chunks
    # are adjacent rows -> one contiguous DRAM range per piece
    piece_out = [
        nc.declare_dram_parameter(f"out{k}", [P, pc[3] - pc[2]], u16,
                                  isOutput=True)
        for k, pc in enumerate(PIECES)
    ]
    piece_out_ap = [t.ap() for t in piece_out]

    shift_and = (mybir.AluOpType.logical_shift_right,
                 mybir.AluOpType.bitwise_and)

    # vector ops: (section, m, lo, hi), ts count = index+1.
    # Section 0 planes 0-1 are split at the rt0/rt1 boundary: the two
    # rt0 halves (gated on in0 alone) absorb receipt divergence between
    # the two input rings, and plane 0 completes at ts3 so the first
    # output piece can go early.
    vops = [(0, 0, 0, W16),                                  # ts 1 (rt0)
            (0, 1, 0, W16),                                  # ts 2 (rt0)
            (0, 0, W16, PW),                                 # ts 3 (rt1)
            (0, 1, W16, PW)]                                 # ts 4 (rt1)
    vops += [(0, m, 0, PW) for m in range(2, PLANES)]        # ts 5..10
    vops += [(1, m, 0, PW) for m in range(PLANES)]           # ts 11..18
    # Queue concurrency does not add bandwidth (HBM write interleaving
    # thrashes); fewer/bigger pieces stream faster, and the SP queue
    # consistently outruns the Act queue (~2x), so all pieces ride the
    # SP queue serially with the ring kept hot (next piece pre-issued).

    from contextlib import ExitStack
    with ExitStack() as ctx:
        xt = [ctx.enter_context(nc.sbuf_tensor(f"xt{s}", [P, PW], u16))
              for s in range(2)]
        ot = [ctx.enter_context(nc.sbuf_tensor(f"ot{s}", [P, OWS], u16))
              for s in range(2)]

        in_sem = [ctx.enter_context(nc.semaphore(f"in_sem{b}"))
                  for b in range(NRT + 1)]   # rt0lo, rt0hi, rt1, rt2, rt3
        ts_sem = ctx.enter_context(nc.semaphore("ts_sem"))
        pts_sem = ctx.enter_context(nc.semaphore("pts_sem"))
        od_sem = ctx.enter_context(nc.semaphore("od_sem"))

        ctx.enter_context(nc.Block(no_gpsimd_drain=True))
        block = nc.cur_block

        @block.vector
        def _(vec: bass.BassEngine):
            for i, (sec, m, lo, hi) in enumerate(vops):
                if i == 0:
                    vec.wait_ge(in_sem[0], 16)
                    vec.wait_ge(in_sem[1], 16)
                elif i == 2:
                    vec.wait_ge(in_sem[2], 16)
                elif i == PLANES + 2:
                    vec.wait_ge(in_sem[3], 16)
                    vec.wait_ge(in_sem[4], 16)
                vec.tensor_scalar(
                    ot[sec][:, m * PW + lo:m * PW + hi],
                    xt[sec][:, lo:hi],
                    7 - m,
                    0x0101,
                    *shift_and,
                ).then_inc(ts_sem)

        def piece_dma(eng, k):
            _, sec, lo, hi, n, pn = PIECES[k]
            eng.wait_ge(ts_sem, n)
            if pn:
                eng.wait_ge(pts_sem, pn)
            eng.dma_start(
                piece_out_ap[k][:, :],
                ot[sec][:, lo:hi],
            ).then_inc(od_sem, 16)

        @block.sync
        def _(sp: bass.BassEngine):
            # rt0 partitions 0-63 (128KB, first on SP ring): splitting
            # rt0 across BOTH HWDGE rings by partition halves halves the
            # transfer time of the gating first input
            sp.dma_start(xt[0][0:P // 2, 0:W16],
                         xin_ap[0:P // 2, :]).then_inc(in_sem[0], 16)
            for k, pc in enumerate(PIECES):
                if pc[0] == 0:
                    piece_dma(sp, k)

        @block.scalar
        def _(sc: bass.BassEngine):
            # rt0 partitions 64-127 (128KB, first on Act ring)
            sc.dma_start(xt[0][P // 2:P, 0:W16],
                         xin_ap[P // 2:P, :]).then_inc(in_sem[1], 16)
            for k, pc in enumerate(PIECES):
                if pc[0] == 1:
                    piece_dma(sc, k)

        @block.gpsimd
        def _(gp: bass.BassEngine):
            # rt1-rt3 via SWDGE, keeping the HWDGE rings free for the
            # gating rt0 halves and the output pieces
            gp.dma_start(xt[0][:, W16:PW],
                         xin_ap[P:2 * P, :]).then_inc(in_sem[2], 16)
            gp.dma_start(xt[1][:, 0:W16],
                         xin_ap[2 * P:3 * P, :]).then_inc(in_sem[3], 16)
            gp.dma_start(xt[1][:, W16:PW],
                         xin_ap[3 * P:4 * P, :]).then_inc(in_sem[4], 16)

    return nc


_NC_CACHE = None


def _get_nc():
    global _NC_CACHE
    if _NC_CACHE is None:
        _NC_CACHE = build_nc()
    return _NC_CACHE


def pack_shard(x_shard: np.ndarray) -> np.ndarray:
    """[ROWS, F] f32 -> [ROWS, W16] uint16: sign-normalized bitcast words
    as a big-endian byte stream, viewed as little-endian uint16 pairs."""
    x_shard = np.ascontiguousarray(x_shard)
    xi = (x_shard.view(np.uint32) & np.uint32(0x7FFFFFFF)) | \
        ((x_shard < 0).astype(np.uint32) << np.uint32(31))
    return xi.byteswap().view(np.uint16)


def unpack_shard(res_map: dict) -> np.ndarray:
    """Per-piece dram tensors -> [ROWS, F, K] f32.

    Pieces are reassembled into the planar pair-section layout
    [P, OCOLS]; section s covers row tiles (2s, 2s+1): bytes [p, sec,
    m, rt_in_pair, 4f+j] -> out[(2*sec+rt)*128+p, f, 8j+m].
    """
    raw = np.empty((P, OCOLS), dtype=np.uint16)
    for k, (eng, sec, lo, hi, n, pn) in enumerate(PIECES):
        raw[:, sec * OWS + lo:sec * OWS + hi] = res_map[f"out{k}"]
    b = raw.view(np.uint8).reshape(P, 2, PLANES, 2, F, 4)
    r = b.transpose(1, 3, 0, 4, 5, 2).reshape(ROWS, F, K)
    return r.astype(np.float32)


def kernel(x: np.ndarray) -> np.ndarray:
    from concourse.bass_utils import run_bass_kernel_spmd

    x = np.asarray(x, dtype=np.float32)
    assert x.shape == (ROWS_TOTAL, F), x.shape
    nc = _get_nc()
    in_maps = [
        {"xin": pack_shard(x[i * ROWS:(i + 1) * ROWS])} for i in range(N_CORES)
    ]
    res = run_bass_kernel_spmd(nc, in_maps, list(range(N_CORES)))
    parts = [unpack_shard(res.results[i]) for i in range(N_CORES)]
    return np.concatenate(parts, axis=0)


# revision 49
# speedup vs baseline: 1.1967x; 1.0380x over previous
"""Trainium2 Bass kernel: float32 -> 32-channel bit-plane encoding.

For input x [4096, 512] f32, produces out [4096, 512, 32] f32 where
out[b, f, 0] = (x[b,f] < 0) and out[b, f, 1+j] = bit (30-j) of
bitcast_int32(|x[b,f]|), MSB first.

Wire-format design: every output element is exactly 0.0 or 1.0, so the
device computes and stores each of the 67M output elements as a uint8
{0,1}; the host applies a value-preserving widening cast to f32.  This
cuts device HBM write traffic 4x (8MB/core instead of 32MB/core), which
is the binding roofline.

Host-side repack makes the device compute uniform:
  i' = (bitcast_u32(x) & 0x7FFFFFFF) | ((x < 0) << 31)
stored as a big-endian byte stream, viewed as uint16 pairs.  Then output
channel k of feature f equals bit (7 - k%8) of stream byte 4f + k//8.

Device compute (VectorE), one fused tensor_scalar op per bit plane:
  plane_m = (x_u16 >> (7-m)) & 0x0101     m = 0..7
Each uint16 element yields TWO planar output bytes; the dense step-1
16-bit single-src pattern hits the DVE 4x perf mode (~602ns steady-state
per FD=2048 op, ~850GB/s production).

Schedule (trace-derived, iterated over ~10 hardware profiles):
  - Concurrent output DMA is throttled to ~310-470GB/s while the DVE
    streams (SBUF port contention) and bursts to ~750-830GB/s after
    the last DVE op, so output pieces are gated as early as each
    plane completes to overlap the throttled phase, while the piece
    count stays at 5 (per-DMA ring/completion overhead is ~1us, and
    finer pieces measurably lower the achieved rate).
  - Queue concurrency does not add bandwidth (the mover pool/AXI port
    is shared and interleaving hurts), so all output pieces ride the
    SP HWDGE queue serially; the ring never empties (each next piece
    is pre-issued), avoiding the ~2.6us cold-restart.
  - Inputs: rt0 on the SP ring, rt1 on the Act ring (both
    first-on-ring, receipt ~2.6-3.3us after issue regardless of size;
    splitting them makes receipt later, not earlier — measured both
    for column splits on one ring and partition splits across rings),
    rt2/rt3 on the GpSimd SWDGE queue.  Section 0 planes 0-1 are
    computed in rt0/rt1 halves so DVE starts on whichever input lands
    first.
  - Each piece writes its own dram tensor (contiguous DRAM range).
    GpSimd SWDGE must not carry output pieces (~77GB/s there), and
    the Pool engine cannot run shift/and tensor_scalar ops.

The planes land in HBM planar per SBUF partition; the host reassembles
pieces and interleaves planes/sections into [rows, F, 32] during the
f32 cast.

Sharded row-wise over 8 NeuronCores (512 rows each, 4 row tiles of 128).
"""

import sys

if "/opt/trn_rl_repo" not in sys.path:
    sys.path.insert(0, "/opt/trn_rl_repo")

import numpy as np

import concourse.bass as bass
import concourse.mybir as mybir

P = 128           # SBUF partitions
F = 512           # features per row
K = 32            # output channels per feature
N_CORES = 8
ROWS_TOTAL = 4096
ROWS = ROWS_TOTAL // N_CORES   # rows per core (512)
NRT = ROWS // P                # row tiles per core (4)
W16 = F * 2                    # uint16 words per row (1024)
HW16 = W16 // 2                # half-row-tile input chunk (512)
PW = 2 * W16                   # pair-section width (2048 u16)
PLANES = 8                     # bit planes per byte
OWS = PLANES * PW              # output u16 per partition per section
OCOLS = 2 * OWS                # output dram columns per partition (32768)
SPLIT = PW - 256               # last-plane split point (u16 cols; the
                               # final op is FD=256 so the last output
                               # piece's gate lands as early as possible)

# out pieces: (engine 0=sync/1=scalar, sec, u16 lo, hi, ts gate,
# pool gate); see build_nc for rationale.  Module-level because the
# host-side unpack reassembles the per-piece dram tensors using the
# same table.
PIECES = [
    (0, 0, 0, 1 * PW, 3, 0),                 # sec0 p0 (512KB)
    (0, 0, 1 * PW, 4 * PW, 7, 0),            # sec0 p1-p3 (1.5MB)
    (0, 0, 4 * PW, 8 * PW, 10, 0),           # sec0 p4-p7 (2MB)
    (0, 1, 0, 4 * PW, 14, 0),                # sec1 p0-p3 (2MB)
    (0, 1, 4 * PW, 8 * PW, 18, 0),           # sec1 p4-p7 (2MB)
]


def build_nc() -> bass.Bass:
    nc = bass.Bass("TRN2", target_bir_lowering=False, debug=False)
    u16 = mybir.dt.uint16

    xin = nc.declare_dram_parameter("xin", [ROWS, W16], u16, isOutput=False)
    xin_ap = xin.ap()
    # one dram tensor per output piece: piece k's 128 partition chunks
    # are adjacent rows -> one contiguous DRAM range per piece
    piece_out = [
        nc.declare_dram_parameter(f"out{k}", [P, pc[3] - pc[2]], u16,
                                  isOutput=True)
        for k, pc in enumerate(PIECES)
    ]
    piece_out_ap = [t.ap() for t in piece_out]

    shift_and = (mybir.AluOpType.logical_shift_right,
                 mybir.AluOpType.bitwise_and)

    # vector ops: (section, m, lo, hi), ts count = index+1.
    # Section 0 planes 0-1 are split at the rt0/rt1 boundary: the two
    # rt0 halves (gated on in0 alone) absorb receipt divergence between
    # the two input rings, and plane 0 completes at ts3 so the first
    # output piece can go early.
    vops = [(0, 0, 0, W16),                                  # ts 1 (rt0)
            (0, 1, 0, W16),                                  # ts 2 (rt0)
            (0, 0, W16, PW),                                 # ts 3 (rt1)
            (0, 1, W16, PW)]                                 # ts 4 (rt1)
    vops += [(0, m, 0, PW) for m in range(2, PLANES)]        # ts 5..10
    vops += [(1, m, 0, PW) for m in range(PLANES)]           # ts 11..18
    # Queue concurrency does not add bandwidth (HBM write interleaving
    # thrashes); fewer/bigger pieces stream faster, and the SP queue
    # consistently outruns the Act queue (~2x), so all pieces ride the
    # SP queue serially with the ring kept hot (next piece pre-issued).

    from contextlib import ExitStack
    with ExitStack() as ctx:
        xt = [ctx.enter_context(nc.sbuf_tensor(f"xt{s}", [P, PW], u16))
              for s in range(2)]
        ot = [ctx.enter_context(nc.sbuf_tensor(f"ot{s}", [P, OWS], u16))
              for s in range(2)]

        in_sem = [ctx.enter_context(nc.semaphore(f"in_sem{b}"))
                  for b in range(NRT)]
        ts_sem = ctx.enter_context(nc.semaphore("ts_sem"))
        pts_sem = ctx.enter_context(nc.semaphore("pts_sem"))
        od_sem = ctx.enter_context(nc.semaphore("od_sem"))

        ctx.enter_context(nc.Block(no_gpsimd_drain=True))
        block = nc.cur_block

        @block.vector
        def _(vec: bass.BassEngine):
            for i, (sec, m, lo, hi) in enumerate(vops):
                if i == 0:
                    vec.wait_ge(in_sem[0], 16)
                elif i == 2:
                    vec.wait_ge(in_sem[1], 16)
                elif i == PLANES + 2:
                    vec.wait_ge(in_sem[2], 16)
                    vec.wait_ge(in_sem[3], 16)
                vec.tensor_scalar(
                    ot[sec][:, m * PW + lo:m * PW + hi],
                    xt[sec][:, lo:hi],
                    7 - m,
                    0x0101,
                    *shift_and,
                ).then_inc(ts_sem)

        def piece_dma(eng, k):
            _, sec, lo, hi, n, pn = PIECES[k]
            eng.wait_ge(ts_sem, n)
            if pn:
                eng.wait_ge(pts_sem, pn)
            eng.dma_start(
                piece_out_ap[k][:, :],
                ot[sec][:, lo:hi],
            ).then_inc(od_sem, 16)

        @block.sync
        def _(sp: bass.BassEngine):
            # rt0 -> xt0 lower half (first and only input on SP ring)
            sp.dma_start(xt[0][:, 0:W16],
                         xin_ap[0:P, :]).then_inc(in_sem[0], 16)
            for k, pc in enumerate(PIECES):
                if pc[0] == 0:
                    piece_dma(sp, k)

        @block.scalar
        def _(sc: bass.BassEngine):
            # rt1 -> xt0 upper half (first and only input on Act ring)
            sc.dma_start(xt[0][:, W16:PW],
                         xin_ap[P:2 * P, :]).then_inc(in_sem[1], 16)
            for k, pc in enumerate(PIECES):
                if pc[0] == 1:
                    piece_dma(sc, k)

        @block.gpsimd
        def _(gp: bass.BassEngine):
            # section 1 inputs via SWDGE, keeping the HWDGE rings free
            # for output pieces
            gp.dma_start(xt[1][:, 0:W16],
                         xin_ap[2 * P:3 * P, :]).then_inc(in_sem[2], 16)
            gp.dma_start(xt[1][:, W16:PW],
                         xin_ap[3 * P:4 * P, :]).then_inc(in_sem[3], 16)

    return nc


_NC_CACHE = None


def _get_nc():
    global _NC_CACHE
    if _NC_CACHE is None:
        _NC_CACHE = build_nc()
    return _NC_CACHE


def pack_shard(x_shard: np.ndarray) -> np.ndarray:
    """[ROWS, F] f32 -> [ROWS, W16] uint16: sign-normalized bitcast words
    as a big-endian byte stream, viewed as little-endian uint16 pairs."""
    x_shard = np.ascontiguousarray(x_shard)
    xi = (x_shard.view(np.uint32) & np.uint32(0x7FFFFFFF)) | \
        ((x_shard < 0).astype(np.uint32) << np.uint32(31))
    return xi.byteswap().view(np.uint16)


def unpack_shard(res_map: dict) -> np.ndarray:
    """Per-piece dram tensors -> [ROWS, F, K] f32.

    Pieces are reassembled into the planar pair-section layout
    [P, OCOLS]; section s covers row tiles (2s, 2s+1): bytes [p, sec,
    m, rt_in_pair, 4f+j] -> out[(2*sec+rt)*128+p, f, 8j+m].
    """
    raw = np.empty((P, OCOLS), dtype=np.uint16)
    for k, (eng, sec, lo, hi, n, pn) in enumerate(PIECES):
        raw[:, sec * OWS + lo:sec * OWS + hi] = res_map[f"out{k}"]
    b = raw.view(np.uint8).reshape(P, 2, PLANES, 2, F, 4)
    r = b.transpose(1, 3, 0, 4, 5, 2).reshape(ROWS, F, K)
    return r.astype(np.float32)


def kernel(x: np.ndarray) -> np.ndarray:
    from concourse.bass_utils import run_bass_kernel_spmd

    x = np.asarray(x, dtype=np.float32)
    assert x.shape == (ROWS_TOTAL, F), x.shape
    nc = _get_nc()
    in_maps = [
        {"xin": pack_shard(x[i * ROWS:(i + 1) * ROWS])} for i in range(N_CORES)
    ]
    res = run_bass_kernel_spmd(nc, in_maps, list(range(N_CORES)))
    parts = [unpack_shard(res.results[i]) for i in range(N_CORES)]
    return np.concatenate(parts, axis=0)
